# revision 4
# baseline (speedup 1.0000x reference)
"""GQA kernel for trn2, 8 cores: DP over batch (2) x TP over kv-head groups (4).

Each core computes, for its (batch b, kv-group g):
  - qkv projection for its 4 q-heads + 1 kv-head (q pre-scaled by 1/sqrt(dk))
  - RoPE on q/k
  - full (non-causal) attention for the 4 q-heads vs its kv-head
  - partial out-projection with its 2048 rows of W_out
Host sums the 4 per-group partials per batch and adds bias.

v2 structure (vs v1):
  - x arrives pre-transposed from host (xt [D, L]) - no on-device transposes
  - all weights DMA'd once and kept resident in SBUF
  - PV runs in the [q-partitions, v-cols] orientation with a ones-column
    appended to v (two N=257 chains per 128-q block), so the softmax
    denominator falls out of the same matmuls that compute P@V - no
    separate 'ones' matmul streams, no [1,512] reciprocal, no broadcast
  - ctx is transposed back to [v, q] on the PE (4 transposes per block,
    hidden) for the fused out-projection

Matmul operands are bf16 (PE runs fp32 at 1/4 rate; bf16 is full rate).
Accumulation and softmax statistics stay fp32.

Self-contained: hardcodes all shapes. kernel(**inputs) -> np.ndarray.
"""

import math
from contextlib import ExitStack

import numpy as np
import ml_dtypes

import concourse.bass as bass
import concourse.bacc as bacc
import concourse.tile as tile
import concourse.mybir as mybir
from concourse.bass_utils import run_bass_kernel_spmd
from concourse.masks import make_identity

F32 = mybir.dt.float32
BF16 = mybir.dt.bfloat16
L = 2048          # sequence length
D = 2048          # d_model
DK = 128          # head dim (q/k)
DV = 512          # head dim (v)
NHQ = 4           # q heads per core
CQK = NHQ * DK + DK   # 640 qk projection cols per core
NT = 16           # d_model chunks of 128
NJ = 16           # key chunks of 128

_NC_CACHE = {}


def build_nc():
    if "nc" in _NC_CACHE:
        return _NC_CACHE["nc"]
    nc = bacc.Bacc("TRN2", target_bir_lowering=False, debug=False)

    xt_d = nc.dram_tensor("xt", [D, L], BF16, kind="ExternalInput")
    wqk_d = nc.dram_tensor("wqk", [D, CQK], BF16, kind="ExternalInput")
    wv_d = nc.dram_tensor("wv", [D, DV], BF16, kind="ExternalInput")
    wo_d = nc.dram_tensor("wo", [NHQ * DV, D], BF16, kind="ExternalInput")
    cos_d = nc.dram_tensor("cost", [DK, L], F32, kind="ExternalInput")
    sin_d = nc.dram_tensor("sint", [DK, L], F32, kind="ExternalInput")
    out_d = nc.dram_tensor("out", [L, D], F32, kind="ExternalOutput")

    EXP = mybir.ActivationFunctionType.Exp

    with ExitStack() as ctx:
        tc = ctx.enter_context(tile.TileContext(nc))
        persist = ctx.enter_context(tc.tile_pool(name="persist", bufs=1))
        pexp = ctx.enter_context(tc.tile_pool(name="pexp", bufs=2))
        # PSUM: 3 (S stream) + 3 (accum chains) + 2 (ctx transposes) = 8 banks
        psS = ctx.enter_context(tc.tile_pool(name="psS", bufs=3, space="PSUM"))
        psPV = ctx.enter_context(tc.tile_pool(name="psPV", bufs=3, space="PSUM"))
        psT = ctx.enter_context(tc.tile_pool(name="psT", bufs=2, space="PSUM"))

        ident = persist.tile([128, 128], BF16)
        make_identity(nc, ident)

        qT = persist.tile([128, NHQ, L], BF16)      # [dk, h, pos]
        kT = persist.tile([128, L], BF16)           # [dk, pos]
        # v with a ones column in the middle: cols 0:256 = v[0:256],
        # col 256 = 1 (softmax denominator rides chain A), 257:513 = v[256:512]
        v_ext = persist.tile([128, NJ, 513], BF16)  # [key_in_chunk, key_chunk, e+]
        nc.vector.memset(v_ext[:, :, 256:257], 1.0)

        expS_tiles = {}

        def alloc_expS():
            t = pexp.tile([128, NJ, 512], BF16, tag="expS", name="expS")
            return t

        def emit_S_block(pair, expS, jlist):
            i, h = pair
            isl = slice(i * 512, (i + 1) * 512)
            for j in jlist:
                ps = psS.tile([128, 512], F32, tag="s")
                nc.tensor.matmul(ps, lhsT=kT[:, j * 128:(j + 1) * 128],
                                 rhs=qT[:, h, isl])
                nc.scalar.activation(out=expS[:, j, :], in_=ps, func=EXP)

        # ---------------- Phase B: qkv projection + rope ----------------
        with tc.tile_pool(name="pb1", bufs=1) as pb1, \
             tc.tile_pool(name="pb2", bufs=2) as pb2:
            xT = pb1.tile([128, NT, L], BF16)
            xr = xt_d.ap().rearrange("(t p) l -> p t l", p=128)
            for nch in range(4):
                sl = slice(nch * 512, (nch + 1) * 512)
                nc.sync.dma_start(out=xT[:, :, sl], in_=xr[:, :, sl])
            wqk_sb = pb1.tile([128, NT, CQK], BF16)
            nc.gpsimd.dma_start(
                out=wqk_sb, in_=wqk_d.ap().rearrange("(t p) c -> p t c", p=128))
            wv_sb = pb1.tile([128, NT, DV], BF16)
            nc.gpsimd.dma_start(
                out=wv_sb, in_=wv_d.ap().rearrange("(t p) c -> p t c", p=128))
            cosT = pb1.tile([128, L], F32)
            sinT = pb1.tile([128, L], F32)
            nc.gpsimd.dma_start(out=cosT, in_=cos_d.ap())
            nc.gpsimd.dma_start(out=sinT, in_=sin_d.ap())

            # q/k projection chains (c = 0..3 q heads, c = 4 is k; k first so
            # attention S for the first pair can interleave with the v chains)
            for c in (4, 0, 1, 2, 3):
                for nch in range(4):
                    ps = psPV.tile([128, 512], F32, tag="pv")
                    for t in range(NT):
                        nc.tensor.matmul(
                            ps, lhsT=wqk_sb[:, t, c * 128:(c + 1) * 128],
                            rhs=xT[:, t, nch * 512:(nch + 1) * 512],
                            start=(t == 0), stop=(t == NT - 1))
                    isl = slice(nch * 512, (nch + 1) * 512)
                    dest = qT[:, c, isl] if c < NHQ else kT[:, isl]
                    cs = cosT[:, isl]
                    sn = sinT[:, isl]
                    tmp = pb2.tile([128, 512], F32, tag="rope")
                    nc.vector.tensor_mul(tmp[0:64, :], ps[64:128, :], sn[0:64, :])
                    nc.vector.tensor_mul(tmp[64:128, :], ps[0:64, :], sn[64:128, :])
                    tmp2 = pb2.tile([128, 512], F32, tag="rope2")
                    nc.vector.tensor_mul(tmp2, ps, cs)
                    nc.vector.tensor_sub(dest[0:64, :], tmp2[0:64, :], tmp[0:64, :])
                    nc.vector.tensor_add(dest[64:128, :], tmp2[64:128, :],
                                         tmp[64:128, :])

            # v projection chains; interleave the first pair's S matmuls so
            # its exp drain overlaps the v chains instead of stalling phase C
            expS_cur = alloc_expS()
            for j in range(NJ):
                ps = psPV.tile([128, 512], F32, tag="pv")
                for t in range(NT):
                    nc.tensor.matmul(
                        ps, lhsT=xT[:, t, j * 128:(j + 1) * 128],
                        rhs=wv_sb[:, t, :],
                        start=(t == 0), stop=(t == NT - 1))
                nc.scalar.copy(out=v_ext[:, j, 0:256], in_=ps[:, 0:256])
                nc.scalar.copy(out=v_ext[:, j, 257:513], in_=ps[:, 256:512])
                if j % 4 == 3:
                    emit_S_block((0, 0), expS_cur, range(j - 3, j + 1))

        # ---------------- Phase C: attention + fused out-projection -----
        with tc.tile_pool(name="pc1", bufs=1) as pc1, \
             tc.tile_pool(name="pc2", bufs=2) as pc2:
            wo_sb = pc1.tile([128, NT, D], BF16)
            wor = wo_d.ap().rearrange("(t p) c -> p t c", p=128)
            for dm in range(4):
                sl = slice(dm * 512, (dm + 1) * 512)
                nc.gpsimd.dma_start(out=wo_sb[:, :, sl], in_=wor[:, :, sl])
            ctxT = [pc1.tile([128, 4, 512], BF16, tag=f"ctxT{h}",
                             name=f"ctxT{h}") for h in range(NHQ)]

            pend = {"T": None}

            def flush_T():
                if pend["T"] is None:
                    return
                h, qc, cxA, cxB = pend["T"]
                pend["T"] = None
                psx = psT.tile([128, 512], BF16, tag="T")
                srcs = (cxA[:, 0:128], cxA[:, 128:256],
                        cxB[:, 0:128], cxB[:, 128:256])
                for ec in range(4):
                    nc.tensor.transpose(psx[:, ec * 128:(ec + 1) * 128],
                                        srcs[ec], ident)
                nc.scalar.copy(
                    out=ctxT[h][:, :, qc * 128:(qc + 1) * 128],
                    in_=psx.rearrange("p (a b) -> p a b", a=4))

            def emit_pv_qc(h, expS, qc):
                qsl = slice(qc * 128, (qc + 1) * 128)
                pa = psPV.tile([128, 512], F32, tag="pv", name="pa")[:, 0:257]
                for j in range(NJ):
                    nc.tensor.matmul(pa, lhsT=expS[:, j, qsl],
                                     rhs=v_ext[:, j, 0:257],
                                     start=(j == 0), stop=(j == NJ - 1))
                pb = psPV.tile([128, 512], F32, tag="pv", name="pb")[:, 0:256]
                for j in range(NJ):
                    nc.tensor.matmul(pb, lhsT=expS[:, j, qsl],
                                     rhs=v_ext[:, j, 257:513],
                                     start=(j == 0), stop=(j == NJ - 1))
                rc = pc2.tile([128, 1], F32, tag="rc")
                nc.vector.reciprocal(rc, pa[:, 256:257])
                cxA = pc2.tile([128, 256], BF16, tag="cxA")
                cxB = pc2.tile([128, 256], BF16, tag="cxB")
                nc.vector.tensor_scalar_mul(cxA, pa[:, 0:256], rc)
                nc.vector.tensor_scalar_mul(cxB, pb[:, 0:256], rc)
                return cxA, cxB

            def emit_outproj(i):
                for dm in range(4):
                    for lsub in range(4):
                        ps = psPV.tile([128, 512], F32, tag="pv")
                        for t2 in range(16):
                            h2, ec = divmod(t2, 4)
                            nc.tensor.matmul(
                                ps,
                                lhsT=ctxT[h2][:, ec,
                                              lsub * 128:(lsub + 1) * 128],
                                rhs=wo_sb[:, t2, dm * 512:(dm + 1) * 512],
                                start=(t2 == 0), stop=(t2 == 15))
                        ost = pc2.tile([128, 512], F32, tag="ost")
                        nc.scalar.copy(out=ost, in_=ps)
                        l0 = i * 512 + lsub * 128
                        nc.sync.dma_start(
                            out=out_d.ap()[l0:l0 + 128,
                                           dm * 512:(dm + 1) * 512],
                            in_=ost)

            pairs = [(i, h) for i in range(4) for h in range(NHQ)]
            for k, (i, h) in enumerate(pairs):
                nxt = pairs[k + 1] if k + 1 < len(pairs) else None
                expS_nxt = alloc_expS() if nxt is not None else None
                for qc in range(4):
                    if nxt is not None:
                        emit_S_block(nxt, expS_nxt, range(qc * 4, qc * 4 + 4))
                    cxA, cxB = emit_pv_qc(h, expS_cur, qc)
                    flush_T()
                    pend["T"] = (h, qc, cxA, cxB)
                if h == NHQ - 1:
                    flush_T()
                    emit_outproj(i)
                expS_cur = expS_nxt

    nc.compile()
    _NC_CACHE["nc"] = nc
    return nc


def make_core_inputs(x, W_attn, W_out):
    """Split full inputs into 8 per-core input maps (core = b*4 + g)."""
    Q_DIM = 2048
    K_DIM = 512
    scale = np.float32(1.0 / math.sqrt(DK))
    bf = ml_dtypes.bfloat16

    # rope tables, mirroring the fp32 reference computation
    inv_freq = (np.float32(1.0) /
                (np.float32(10000.0) **
                 (np.arange(0, DK, 2, dtype=np.float32) / np.float32(DK))))
    freqs = np.arange(L, dtype=np.float32)[:, None] * inv_freq[None, :]  # [L,64]
    ang = np.concatenate([freqs, freqs], axis=-1)  # [L, 128]
    cosT = np.ascontiguousarray(np.cos(ang).T.astype(np.float32))  # [128, L]
    sinT = np.ascontiguousarray(np.sin(ang).T.astype(np.float32))

    xts = [np.ascontiguousarray(x[b].T).astype(bf) for b in range(2)]

    in_maps = []
    for core in range(8):
        b, g = divmod(core, 4)
        wq = (W_attn[:, 512 * g:512 * (g + 1)] * scale)
        wk = W_attn[:, Q_DIM + 128 * g:Q_DIM + 128 * (g + 1)]
        wqk = np.ascontiguousarray(
            np.concatenate([wq, wk], axis=1)).astype(bf)
        wv = np.ascontiguousarray(W_attn[:, Q_DIM + K_DIM + 512 * g:
                                         Q_DIM + K_DIM + 512 * (g + 1)]).astype(bf)
        wo = np.ascontiguousarray(W_out[2048 * g:2048 * (g + 1), :]).astype(bf)
        in_maps.append({
            "xt": xts[b],
            "wqk": wqk,
            "wv": wv,
            "wo": wo,
            "cost": cosT,
            "sint": sinT,
        })
    return in_maps


def kernel(x, W_attn, W_out, b_out, _trace=False, _trace_cores=None):
    x = np.asarray(x)
    W_attn = np.asarray(W_attn)
    W_out = np.asarray(W_out)
    b_out = np.asarray(b_out)
    nc = build_nc()
    in_maps = make_core_inputs(x, W_attn, W_out)
    res = run_bass_kernel_spmd(
        nc, in_maps, core_ids=list(range(8)),
        trace=_trace, trace_cores=_trace_cores)
    parts = [res.results[c]["out"] for c in range(8)]
    out = np.empty((2, L, D), dtype=np.float32)
    for b in range(2):
        acc = parts[4 * b].astype(np.float32)
        for g in range(1, 4):
            acc = acc + parts[4 * b + g]
        out[b] = acc + b_out[None, :].astype(np.float32)
    if _trace:
        kernel._last_results = res
    return out


# revision 6
# speedup vs baseline: 1.2185x; 1.2185x over previous
"""GQA kernel for trn2, 8 cores: DP over batch (2) x TP over kv-head groups (4).

Each core computes, for its (batch b, kv-group g):
  - qkv projection for its 4 q-heads + 1 kv-head (q pre-scaled by 1/sqrt(dk))
  - RoPE on q/k
  - full (non-causal) attention for the 4 q-heads vs its kv-head
  - partial out-projection with its 2048 rows of W_out
Host sums the 4 per-group partials per batch and adds bias.

v2 structure (vs v1):
  - x arrives pre-transposed from host (xt [D, L]) - no on-device transposes
  - all weights DMA'd once and kept resident in SBUF
  - PV runs in the [q-partitions, v-cols] orientation with a ones-column
    appended to v (two N=257 chains per 128-q block), so the softmax
    denominator falls out of the same matmuls that compute P@V - no
    separate 'ones' matmul streams, no [1,512] reciprocal, no broadcast
  - ctx is transposed back to [v, q] on the PE (4 transposes per block,
    hidden) for the fused out-projection

Matmul operands are bf16 (PE runs fp32 at 1/4 rate; bf16 is full rate).
Accumulation and softmax statistics stay fp32.

Self-contained: hardcodes all shapes. kernel(**inputs) -> np.ndarray.
"""

import math
from contextlib import ExitStack

import numpy as np
import ml_dtypes

import concourse.bass as bass
import concourse.bacc as bacc
import concourse.tile as tile
import concourse.mybir as mybir
from concourse.bass_utils import run_bass_kernel_spmd
from concourse.masks import make_identity

F32 = mybir.dt.float32
BF16 = mybir.dt.bfloat16
L = 2048          # sequence length
D = 2048          # d_model
DK = 128          # head dim (q/k)
DV = 512          # head dim (v)
NHQ = 4           # q heads per core
CQK = NHQ * DK + DK   # 640 qk projection cols per core
NT = 16           # d_model chunks of 128
NJ = 16           # key chunks of 128

_NC_CACHE = {}


def build_nc():
    if "nc" in _NC_CACHE:
        return _NC_CACHE["nc"]
    nc = bacc.Bacc("TRN2", target_bir_lowering=False, debug=False)

    xt_d = nc.dram_tensor("xt", [D, L], BF16, kind="ExternalInput")
    wqk_d = nc.dram_tensor("wqk", [D, CQK], BF16, kind="ExternalInput")
    wv_d = nc.dram_tensor("wv", [D, DV], BF16, kind="ExternalInput")
    wo_d = nc.dram_tensor("wo", [NHQ * DV, D], BF16, kind="ExternalInput")
    cos_d = nc.dram_tensor("cost", [DK, L], F32, kind="ExternalInput")
    sin_d = nc.dram_tensor("sint", [DK, L], F32, kind="ExternalInput")
    out_d = nc.dram_tensor("out", [L, D], F32, kind="ExternalOutput")

    EXP = mybir.ActivationFunctionType.Exp

    with ExitStack() as ctx:
        tc = ctx.enter_context(tile.TileContext(nc))
        persist = ctx.enter_context(tc.tile_pool(name="persist", bufs=1))
        pexp = ctx.enter_context(tc.tile_pool(name="pexp", bufs=2))
        # PSUM: 3 (S stream) + 3 (accum chains) + 2 (ctx transposes) = 8 banks
        psS = ctx.enter_context(tc.tile_pool(name="psS", bufs=3, space="PSUM"))
        psPV = ctx.enter_context(tc.tile_pool(name="psPV", bufs=3, space="PSUM"))
        psT = ctx.enter_context(tc.tile_pool(name="psT", bufs=2, space="PSUM"))

        ident = persist.tile([128, 128], BF16)
        make_identity(nc, ident)

        qT = persist.tile([128, NHQ, L], BF16)      # [dk, h, pos]
        kT = persist.tile([128, L], BF16)           # [dk, pos]
        # v with a ones column in the middle: cols 0:256 = v[0:256],
        # col 256 = 1 (softmax denominator rides chain A), 257:513 = v[256:512]
        v_ext = persist.tile([128, NJ, 513], BF16)  # [key_in_chunk, key_chunk, e+]
        nc.vector.memset(v_ext[:, :, 256:257], 1.0)

        expS_tiles = {}

        def alloc_expS():
            t = pexp.tile([128, NJ, 512], BF16, tag="expS", name="expS")
            return t

        def emit_S_block(pair, expS, jlist):
            i, h = pair
            isl = slice(i * 512, (i + 1) * 512)
            for j in jlist:
                ps = psS.tile([128, 512], F32, tag="s")
                nc.tensor.matmul(ps, lhsT=kT[:, j * 128:(j + 1) * 128],
                                 rhs=qT[:, h, isl])
                nc.scalar.activation(out=expS[:, j, :], in_=ps, func=EXP)

        # ---------------- Phase B: qkv projection + rope ----------------
        with tc.tile_pool(name="pb1", bufs=1) as pb1, \
             tc.tile_pool(name="pb2", bufs=2) as pb2:
            xT = pb1.tile([128, NT, L], BF16)
            xr = xt_d.ap().rearrange("(t p) l -> p t l", p=128)
            for nch in range(4):
                sl = slice(nch * 512, (nch + 1) * 512)
                nc.sync.dma_start(out=xT[:, :, sl], in_=xr[:, :, sl])
            wqk_sb = pb1.tile([128, NT, CQK], BF16)
            wqr = wqk_d.ap().rearrange("(t p) c -> p t c", p=128)
            # k-head slice first: the first chain (c=4) only needs these cols
            nc.gpsimd.dma_start(out=wqk_sb[:, :, 512:640], in_=wqr[:, :, 512:640])
            nc.gpsimd.dma_start(out=wqk_sb[:, :, 0:512], in_=wqr[:, :, 0:512])
            wv_sb = pb1.tile([128, NT, DV], BF16)
            nc.gpsimd.dma_start(
                out=wv_sb, in_=wv_d.ap().rearrange("(t p) c -> p t c", p=128))
            cosT = pb1.tile([128, L], F32)
            sinT = pb1.tile([128, L], F32)
            nc.gpsimd.dma_start(out=cosT, in_=cos_d.ap())
            nc.gpsimd.dma_start(out=sinT, in_=sin_d.ap())

            # q/k projection chains (c = 0..3 q heads, c = 4 is k).
            # nch-outer: the 5 chains of one nch group all stream the same
            # 2MB xT chunk, so the PE consumes HBM at a sustainable rate
            # during startup instead of outrunning the xT DMA 5x.
            for nch in range(4):
                for c in (4, 0, 1, 2, 3):
                    ps = psPV.tile([128, 512], F32, tag="pv")
                    for t in range(NT):
                        nc.tensor.matmul(
                            ps, lhsT=wqk_sb[:, t, c * 128:(c + 1) * 128],
                            rhs=xT[:, t, nch * 512:(nch + 1) * 512],
                            start=(t == 0), stop=(t == NT - 1))
                    isl = slice(nch * 512, (nch + 1) * 512)
                    dest = qT[:, c, isl] if c < NHQ else kT[:, isl]
                    cs = cosT[:, isl]
                    sn = sinT[:, isl]
                    tmp = pb2.tile([128, 512], F32, tag="rope")
                    nc.vector.tensor_mul(tmp[0:64, :], ps[64:128, :], sn[0:64, :])
                    nc.vector.tensor_mul(tmp[64:128, :], ps[0:64, :], sn[64:128, :])
                    tmp2 = pb2.tile([128, 512], F32, tag="rope2")
                    nc.vector.tensor_mul(tmp2, ps, cs)
                    nc.vector.tensor_sub(dest[0:64, :], tmp2[0:64, :], tmp[0:64, :])
                    nc.vector.tensor_add(dest[64:128, :], tmp2[64:128, :],
                                         tmp[64:128, :])

            # v projection chains; interleave the first pair's S matmuls so
            # its exp drain overlaps the v chains instead of stalling phase C
            expS_cur = alloc_expS()
            for j in range(NJ):
                ps = psPV.tile([128, 512], F32, tag="pv")
                for t in range(NT):
                    nc.tensor.matmul(
                        ps, lhsT=xT[:, t, j * 128:(j + 1) * 128],
                        rhs=wv_sb[:, t, :],
                        start=(t == 0), stop=(t == NT - 1))
                nc.scalar.copy(out=v_ext[:, j, 0:256], in_=ps[:, 0:256])
                nc.scalar.copy(out=v_ext[:, j, 257:513], in_=ps[:, 256:512])
                if j % 4 == 3:
                    emit_S_block((0, 0), expS_cur, range(j - 3, j + 1))

        # ---------------- Phase C: attention + fused out-projection -----
        with tc.tile_pool(name="pc1", bufs=1) as pc1, \
             tc.tile_pool(name="pc2", bufs=2) as pc2:
            wo_sb = pc1.tile([128, NT, D], BF16)
            wor = wo_d.ap().rearrange("(t p) c -> p t c", p=128)
            for dm in range(4):
                sl = slice(dm * 512, (dm + 1) * 512)
                nc.gpsimd.dma_start(out=wo_sb[:, :, sl], in_=wor[:, :, sl])
            ctxT = [pc1.tile([128, 4, 512], BF16, tag=f"ctxT{h}",
                             name=f"ctxT{h}") for h in range(NHQ)]

            pend = {"T": None}

            def flush_T():
                if pend["T"] is None:
                    return
                h, qc, cxA, cxB = pend["T"]
                pend["T"] = None
                psx = psT.tile([128, 512], BF16, tag="T")
                srcs = (cxA[:, 0:128], cxA[:, 128:256],
                        cxB[:, 0:128], cxB[:, 128:256])
                for ec in range(4):
                    nc.tensor.transpose(psx[:, ec * 128:(ec + 1) * 128],
                                        srcs[ec], ident)
                nc.scalar.copy(
                    out=ctxT[h][:, :, qc * 128:(qc + 1) * 128],
                    in_=psx.rearrange("p (a b) -> p a b", a=4))

            def emit_pv_qc(h, expS, qc):
                qsl = slice(qc * 128, (qc + 1) * 128)
                pa = psPV.tile([128, 512], F32, tag="pv", name="pa")[:, 0:257]
                for j in range(NJ):
                    nc.tensor.matmul(pa, lhsT=expS[:, j, qsl],
                                     rhs=v_ext[:, j, 0:257],
                                     start=(j == 0), stop=(j == NJ - 1))
                pb = psPV.tile([128, 512], F32, tag="pv", name="pb")[:, 0:256]
                for j in range(NJ):
                    nc.tensor.matmul(pb, lhsT=expS[:, j, qsl],
                                     rhs=v_ext[:, j, 257:513],
                                     start=(j == 0), stop=(j == NJ - 1))
                rc = pc2.tile([128, 1], F32, tag="rc")
                nc.vector.reciprocal(rc, pa[:, 256:257])
                cxA = pc2.tile([128, 256], BF16, tag="cxA")
                cxB = pc2.tile([128, 256], BF16, tag="cxB")
                nc.vector.tensor_scalar_mul(cxA, pa[:, 0:256], rc)
                nc.vector.tensor_scalar_mul(cxB, pb[:, 0:256], rc)
                return cxA, cxB

            def emit_outproj(i):
                for dm in range(4):
                    for lsub in range(4):
                        ps = psPV.tile([128, 512], F32, tag="pv")
                        for t2 in range(16):
                            h2, ec = divmod(t2, 4)
                            nc.tensor.matmul(
                                ps,
                                lhsT=ctxT[h2][:, ec,
                                              lsub * 128:(lsub + 1) * 128],
                                rhs=wo_sb[:, t2, dm * 512:(dm + 1) * 512],
                                start=(t2 == 0), stop=(t2 == 15))
                        ost = pc2.tile([128, 512], F32, tag="ost")
                        nc.scalar.copy(out=ost, in_=ps)
                        l0 = i * 512 + lsub * 128
                        nc.sync.dma_start(
                            out=out_d.ap()[l0:l0 + 128,
                                           dm * 512:(dm + 1) * 512],
                            in_=ost)

            pairs = [(i, h) for i in range(4) for h in range(NHQ)]
            for k, (i, h) in enumerate(pairs):
                nxt = pairs[k + 1] if k + 1 < len(pairs) else None
                expS_nxt = alloc_expS() if nxt is not None else None
                for qc in range(4):
                    if nxt is not None:
                        emit_S_block(nxt, expS_nxt, range(qc * 4, qc * 4 + 4))
                    cxA, cxB = emit_pv_qc(h, expS_cur, qc)
                    flush_T()
                    pend["T"] = (h, qc, cxA, cxB)
                if h == NHQ - 1:
                    flush_T()
                    emit_outproj(i)
                expS_cur = expS_nxt

    nc.compile()
    _NC_CACHE["nc"] = nc
    return nc


def make_core_inputs(x, W_attn, W_out):
    """Split full inputs into 8 per-core input maps (core = b*4 + g)."""
    Q_DIM = 2048
    K_DIM = 512
    scale = np.float32(1.0 / math.sqrt(DK))
    bf = ml_dtypes.bfloat16

    # rope tables, mirroring the fp32 reference computation
    inv_freq = (np.float32(1.0) /
                (np.float32(10000.0) **
                 (np.arange(0, DK, 2, dtype=np.float32) / np.float32(DK))))
    freqs = np.arange(L, dtype=np.float32)[:, None] * inv_freq[None, :]  # [L,64]
    ang = np.concatenate([freqs, freqs], axis=-1)  # [L, 128]
    cosT = np.ascontiguousarray(np.cos(ang).T.astype(np.float32))  # [128, L]
    sinT = np.ascontiguousarray(np.sin(ang).T.astype(np.float32))

    xts = [np.ascontiguousarray(x[b].T).astype(bf) for b in range(2)]

    in_maps = []
    for core in range(8):
        b, g = divmod(core, 4)
        wq = (W_attn[:, 512 * g:512 * (g + 1)] * scale)
        wk = W_attn[:, Q_DIM + 128 * g:Q_DIM + 128 * (g + 1)]
        wqk = np.ascontiguousarray(
            np.concatenate([wq, wk], axis=1)).astype(bf)
        wv = np.ascontiguousarray(W_attn[:, Q_DIM + K_DIM + 512 * g:
                                         Q_DIM + K_DIM + 512 * (g + 1)]).astype(bf)
        wo = np.ascontiguousarray(W_out[2048 * g:2048 * (g + 1), :]).astype(bf)
        in_maps.append({
            "xt": xts[b],
            "wqk": wqk,
            "wv": wv,
            "wo": wo,
            "cost": cosT,
            "sint": sinT,
        })
    return in_maps


def kernel(x, W_attn, W_out, b_out, _trace=False, _trace_cores=None):
    x = np.asarray(x)
    W_attn = np.asarray(W_attn)
    W_out = np.asarray(W_out)
    b_out = np.asarray(b_out)
    nc = build_nc()
    in_maps = make_core_inputs(x, W_attn, W_out)
    res = run_bass_kernel_spmd(
        nc, in_maps, core_ids=list(range(8)),
        trace=_trace, trace_cores=_trace_cores)
    parts = [res.results[c]["out"] for c in range(8)]
    out = np.empty((2, L, D), dtype=np.float32)
    for b in range(2):
        acc = parts[4 * b].astype(np.float32)
        for g in range(1, 4):
            acc = acc + parts[4 * b + g]
        out[b] = acc + b_out[None, :].astype(np.float32)
    if _trace:
        kernel._last_results = res
    return out
